# revision 1
# baseline (speedup 1.0000x reference)
"""ComplEx scoring kernel for Trainium2, sharded over 8 NeuronCores.

Computes: result[b, e] = tmp1[b] . E_im[e] + tmp2[b] . E_re[e] + mask[e]
where tmp1/tmp2 are complex-product combinations of gathered entity and
relation embeddings (with inverse-relation sign handling).

Sharding: entity dimension (100000) split across 8 cores (12500 each).
Batch and relation tables replicated. Each core redundantly computes the
gathered tmp1/tmp2 (needs the full entity table for the gather), then
GEMMs against its own entity shard and emits [1024, 12500] logits.
"""

import sys

sys.path.insert(0, "/opt/trn_rl_repo")

import numpy as np

import concourse.bacc as bacc
import concourse.bass as bass
import concourse.mybir as mybir
import concourse.tile as tile
from concourse.bass import IndirectOffsetOnAxis
from concourse.bass_utils import run_bass_kernel_spmd
from concourse.masks import make_identity

F32 = mybir.dt.float32
F32R = mybir.dt.float32r
I32 = mybir.dt.int32

NUM_ENTITIES = 100000
DIM = 512
BATCH = 1024
NUM_REL = 500  # NUM_REL_TOTAL // 2
N_CORES = 8
ESH = NUM_ENTITIES // N_CORES  # 12500 entities per core
ET = 500                       # entity tile (free dim of each matmul)
N_ET = ESH // ET               # 25 e-tiles per core
NB = BATCH // 128              # 8 batch tiles
NC_D = DIM // 128              # 4 contraction chunks per table


def build_module(
    use_f32r=True,
    nrep=1,
    do_prologue=True,   # gathers + elementwise + transposes (else memset tmpT)
    do_rhs_dma=True,    # stream rhs tiles from DRAM (else one static tile)
    do_mm=True,         # matmuls
    store_all=True,     # all output stores (else only et==0)
):
    nc = bacc.Bacc("TRN2", target_bir_lowering=False, debug=False)

    hix = nc.dram_tensor("hix", [128, NB], I32, kind="ExternalInput")
    rix = nc.dram_tensor("rix", [128, NB], I32, kind="ExternalInput")
    eim_full = nc.dram_tensor("eim_full", [NUM_ENTITIES, DIM], F32, kind="ExternalInput")
    ere_full = nc.dram_tensor("ere_full", [NUM_ENTITIES, DIM], F32, kind="ExternalInput")
    eimT = nc.dram_tensor("eimT", [DIM, ESH], F32, kind="ExternalInput")
    ereT = nc.dram_tensor("ereT", [DIM, ESH], F32, kind="ExternalInput")
    rim = nc.dram_tensor("rim", [NUM_REL, DIM], F32, kind="ExternalInput")
    rre = nc.dram_tensor("rre", [NUM_REL, DIM], F32, kind="ExternalInput")
    maskrep = nc.dram_tensor("maskrep", [128, ESH], F32, kind="ExternalInput")
    out = nc.dram_tensor("out", [BATCH, ESH], F32, kind="ExternalOutput")

    mm_dt = F32R if use_f32r else F32

    with tile.TileContext(nc) as tc:
        with (
            tc.tile_pool(name="cpool", bufs=1) as cpool,
            tc.tile_pool(name="gpool", bufs=3) as gpool,
            tc.tile_pool(name="epool", bufs=3) as epool,
            tc.tile_pool(name="persist", bufs=1) as ppool,
            tc.tile_pool(name="tps", bufs=4, space="PSUM") as tpsum,
            tc.tile_pool(name="rhspool", bufs=2) as rhspool,
            tc.tile_pool(name="mpool", bufs=2) as mpool,
            tc.tile_pool(name="outpool", bufs=4) as outpool,
            tc.tile_pool(name="psum", bufs=4, space="PSUM") as psum,
        ):
          for _rep in range(nrep):
            # ---- constants / index preprocessing (on device) ----
            identity = cpool.tile([128, 128], F32)
            make_identity(nc, identity[:])

            hix_sb = cpool.tile([128, NB], I32)
            nc.sync.dma_start(hix_sb[:], hix[:])
            rix_sb = cpool.tile([128, NB], I32)
            nc.sync.dma_start(rix_sb[:], rix[:])

            rf = cpool.tile([128, NB], F32)
            nc.vector.tensor_copy(rf[:], rix_sb[:])
            ge = cpool.tile([128, NB], F32)
            nc.vector.tensor_scalar(
                ge[:], rf[:], float(NUM_REL) - 0.5, None, op0=mybir.AluOpType.is_gt
            )
            # sign s = 1 - 2*[r >= NUM_REL]
            sall = cpool.tile([128, NB], F32)
            nc.vector.tensor_scalar(
                sall[:], ge[:], -2.0, 1.0,
                op0=mybir.AluOpType.mult, op1=mybir.AluOpType.add,
            )
            # r_eff = r - NUM_REL*[r >= NUM_REL]
            ge500 = cpool.tile([128, NB], F32)
            nc.vector.tensor_scalar(
                ge500[:], ge[:], float(NUM_REL), None, op0=mybir.AluOpType.mult
            )
            reff_f = cpool.tile([128, NB], F32)
            nc.vector.tensor_sub(reff_f[:], rf[:], ge500[:])
            reff = cpool.tile([128, NB], I32)
            nc.vector.tensor_copy(reff[:], reff_f[:])

            # ---- gather + elementwise + transpose: build tmp1T/tmp2T ----
            # tmp{1,2}T layout: [128 (d within chunk), NB*DIM] where column
            # bt*DIM + c*128 + j holds tmp[bt*128 + j, c*128 + d]
            tmp1T = [ppool.tile([128, DIM], mm_dt, tag=f"t1T{b}", name=f"t1T{b}") for b in range(NB)]
            tmp2T = [ppool.tile([128, DIM], mm_dt, tag=f"t2T{b}", name=f"t2T{b}") for b in range(NB)]

            if not do_prologue:
                scratch = cpool.tile([128, DIM], F32)
                nc.vector.memset(scratch[:], 0.001)
                for b in range(NB):
                    nc.vector.tensor_copy(tmp1T[b][:], scratch[:])
                    nc.vector.tensor_copy(tmp2T[b][:], scratch[:])
            for bt in range(NB if do_prologue else 0):
                h_im = gpool.tile([128, DIM], F32, tag="h_im")
                nc.gpsimd.indirect_dma_start(
                    out=h_im[:], out_offset=None, in_=eim_full[:],
                    in_offset=IndirectOffsetOnAxis(ap=hix_sb[:, bt : bt + 1], axis=0),
                )
                h_re = gpool.tile([128, DIM], F32, tag="h_re")
                nc.gpsimd.indirect_dma_start(
                    out=h_re[:], out_offset=None, in_=ere_full[:],
                    in_offset=IndirectOffsetOnAxis(ap=hix_sb[:, bt : bt + 1], axis=0),
                )
                r_im = gpool.tile([128, DIM], F32, tag="r_im")
                nc.gpsimd.indirect_dma_start(
                    out=r_im[:], out_offset=None, in_=rim[:],
                    in_offset=IndirectOffsetOnAxis(ap=reff[:, bt : bt + 1], axis=0),
                )
                r_re = gpool.tile([128, DIM], F32, tag="r_re")
                nc.gpsimd.indirect_dma_start(
                    out=r_re[:], out_offset=None, in_=rre[:],
                    in_offset=IndirectOffsetOnAxis(ap=reff[:, bt : bt + 1], axis=0),
                )

                # r_im' = s * r_im  (per-partition scalar)
                rimp = epool.tile([128, DIM], F32, tag="rimp")
                nc.vector.tensor_scalar(
                    rimp[:], r_im[:], sall[:, bt : bt + 1], None,
                    op0=mybir.AluOpType.mult,
                )
                # tmp1 = h_im*r_re + h_re*r_im'
                pa = epool.tile([128, DIM], F32, tag="pa")
                nc.vector.tensor_mul(pa[:], h_im[:], r_re[:])
                pb = epool.tile([128, DIM], F32, tag="pb")
                nc.vector.tensor_mul(pb[:], h_re[:], rimp[:])
                tmp1 = epool.tile([128, DIM], F32, tag="tmp1")
                nc.vector.tensor_add(tmp1[:], pa[:], pb[:])
                # tmp2 = h_re*r_re - h_im*r_im'
                pc = epool.tile([128, DIM], F32, tag="pc")
                nc.vector.tensor_mul(pc[:], h_re[:], r_re[:])
                pd = epool.tile([128, DIM], F32, tag="pd")
                nc.vector.tensor_mul(pd[:], h_im[:], rimp[:])
                tmp2 = epool.tile([128, DIM], F32, tag="tmp2")
                nc.vector.tensor_sub(tmp2[:], pc[:], pd[:])

                for src, dst in ((tmp1, tmp1T[bt]), (tmp2, tmp2T[bt])):
                    for c in range(NC_D):
                        pt = tpsum.tile([128, 128], F32, tag="pt")
                        nc.tensor.transpose(
                            pt[:], src[:, c * 128 : (c + 1) * 128], identity[:]
                        )
                        nc.vector.tensor_copy(
                            dst[:, c * 128 : (c + 1) * 128], pt[:]
                        )

            # ---- main GEMM: out[b, e] = tmp1 @ E_im^T + tmp2 @ E_re^T + mask ----
            if not do_rhs_dma:
                rhs_static = rhspool.tile([128, 2 * NC_D * ET], mm_dt, tag="rhss")
                scratch2 = cpool.tile([128, 2 * NC_D * ET], F32)
                nc.vector.memset(scratch2[:], 0.001)
                nc.vector.tensor_copy(rhs_static[:], scratch2[:])
            for et in range(N_ET):
                e0 = et * ET
                if do_rhs_dma:
                    rhs = rhspool.tile([128, 2 * NC_D * ET], mm_dt, tag="rhs")
                    for t, eT in enumerate((eimT, ereT)):
                        for c in range(NC_D):
                            # SWDGE (gpsimd) casts f32 -> f32r during the DMA;
                            # plain HWDGE path when matmuls run in plain f32.
                            dma_eng = nc.gpsimd if use_f32r else nc.sync
                            dma_eng.dma_start(
                                rhs[:, (t * NC_D + c) * ET : (t * NC_D + c + 1) * ET],
                                eT[c * 128 : (c + 1) * 128, e0 : e0 + ET],
                            )
                else:
                    rhs = rhs_static
                mtile = mpool.tile([128, ET], F32, tag="mtile")
                nc.sync.dma_start(mtile[:], maskrep[:, e0 : e0 + ET])

                for bt in range(NB):
                    store = store_all or et == 0
                    if not do_mm:
                        continue
                    ps = psum.tile([128, ET], F32, tag="ps")
                    k = 0
                    for t, tT in enumerate((tmp1T, tmp2T)):
                        for c in range(NC_D):
                            nc.tensor.matmul(
                                ps[:],
                                lhsT=tT[bt][:, c * 128 : (c + 1) * 128],
                                rhs=rhs[
                                    :, (t * NC_D + c) * ET : (t * NC_D + c + 1) * ET
                                ],
                                start=(k == 0),
                                stop=(k == 2 * NC_D - 1),
                            )
                            k += 1
                    if store:
                        ot = outpool.tile([128, ET], F32, tag="ot")
                        nc.vector.tensor_add(ot[:], ps[:], mtile[:])
                        nc.sync.dma_start(
                            out[bt * 128 : (bt + 1) * 128, e0 : e0 + ET], ot[:]
                        )

    nc.compile()
    return nc


_NC_CACHE = {}


def _get_module(use_f32r=True):
    key = use_f32r
    if key not in _NC_CACHE:
        _NC_CACHE[key] = build_module(use_f32r)
    return _NC_CACHE[key]


def make_in_maps(h, r, E_im, E_re, R_im, R_re, mask):
    """Host-side sharding / layout prep (value-independent transforms only)."""
    h32 = np.ascontiguousarray(np.asarray(h, dtype=np.int32).reshape(NB, 128).T)
    r32 = np.ascontiguousarray(np.asarray(r, dtype=np.int32).reshape(NB, 128).T)
    E_im = np.asarray(E_im, dtype=np.float32)
    E_re = np.asarray(E_re, dtype=np.float32)
    rim = np.ascontiguousarray(np.asarray(R_im, dtype=np.float32)[:NUM_REL])
    rre = np.ascontiguousarray(np.asarray(R_re, dtype=np.float32)[:NUM_REL])
    mask = np.asarray(mask, dtype=np.float32).reshape(1, NUM_ENTITIES)

    in_maps = []
    for k in range(N_CORES):
        sl = slice(k * ESH, (k + 1) * ESH)
        in_maps.append(
            {
                "hix": h32,
                "rix": r32,
                "eim_full": E_im,
                "ere_full": E_re,
                "eimT": np.ascontiguousarray(E_im[sl].T),
                "ereT": np.ascontiguousarray(E_re[sl].T),
                "rim": rim,
                "rre": rre,
                "maskrep": np.ascontiguousarray(
                    np.broadcast_to(mask[:, sl], (128, ESH))
                ),
            }
        )
    return in_maps


def kernel(h, r, E_im, E_re, R_im, R_re, mask):
    nc = _get_module()
    in_maps = make_in_maps(h, r, E_im, E_re, R_im, R_re, mask)
    res = run_bass_kernel_spmd(nc, in_maps, core_ids=list(range(N_CORES)))
    return np.concatenate([res.results[k]["out"] for k in range(N_CORES)], axis=1)



# revision 3
# speedup vs baseline: 1.5300x; 1.5300x over previous
"""ComplEx scoring kernel for Trainium2, sharded over 8 NeuronCores.

Computes: result[b, e] = tmp1[b] . E_im[e] + tmp2[b] . E_re[e] + mask[e]
where tmp1/tmp2 are complex-product combinations of gathered entity and
relation embeddings (with inverse-relation sign handling).

Sharding: entity dimension (100000) split across 8 cores (12500 each).
Batch and relation tables replicated. Each core redundantly computes the
gathered tmp1/tmp2 (needs the full entity table for the gather), then
GEMMs against its own entity shard and emits [1024, 12500] logits.

v2: GEMM operands in bf16 (host-pre-formatted entity shard, device-cast
tmp), all streaming DMAs on the HWDGE path with per-partition-contiguous
tiled layout, gathers from host-concatenated [E_im|E_re] / [R_im|R_re]
tables (one 4KB row per index), and a two-phase head group order so the
matmul pipeline never waits on late-batch prologue work.
"""

import sys

sys.path.insert(0, "/opt/trn_rl_repo")

import numpy as np

import concourse.bacc as bacc
import concourse.bass as bass
import concourse.mybir as mybir
import concourse.tile as tile
from concourse.bass import IndirectOffsetOnAxis
from concourse.bass_utils import run_bass_kernel_spmd
from concourse.masks import make_identity

F32 = mybir.dt.float32
F32R = mybir.dt.float32r
BF16 = mybir.dt.bfloat16
I32 = mybir.dt.int32

NUM_ENTITIES = 100000
DIM = 512
BATCH = 1024
NUM_REL = 500  # NUM_REL_TOTAL // 2
N_CORES = 8
ESH = NUM_ENTITIES // N_CORES  # 12500 entities per core
ET = 500                       # entity tile (free dim of each matmul)
N_ET = ESH // ET               # 25 e-tiles per core
NB = BATCH // 128              # 8 batch tiles
KCH = 2 * DIM // 128           # 8 contraction chunks over [tmp1|tmp2]
NHEAD = 6                      # e-tiles kept resident for the two-phase head


def build_module(
    dtype="bf16",
    nrep=1,
    nhead=NHEAD,
    do_prologue=True,   # gathers + elementwise + transposes (else memset tmpT)
    do_rhs_dma=True,    # stream rhs tiles from DRAM (else one static tile)
    do_mm=True,         # matmuls
    store_all=True,     # all output stores (else only et==0)
):
    mm_dt = {"bf16": BF16, "f32r": F32R}[dtype]
    mm_sz = 2 if dtype == "bf16" else 4
    nc = bacc.Bacc("TRN2", target_bir_lowering=False, debug=False)

    hix = nc.dram_tensor("hix", [128, NB], I32, kind="ExternalInput")
    rix = nc.dram_tensor("rix", [128, NB], I32, kind="ExternalInput")
    ec = nc.dram_tensor("ec", [NUM_ENTITIES, 2 * DIM], F32, kind="ExternalInput")
    rc = nc.dram_tensor("rc", [NUM_REL, 2 * DIM], F32, kind="ExternalInput")
    ecatT = nc.dram_tensor("ecatT", [128, N_ET * KCH * ET], mm_dt, kind="ExternalInput")
    maskrep = nc.dram_tensor("maskrep", [128, ESH], F32, kind="ExternalInput")
    out = nc.dram_tensor("out", [BATCH, ESH], F32, kind="ExternalOutput")

    # rhs pool must keep the head e-tiles resident for the bt 4-7 revisit
    rhs_bufs = (nhead + 2) if nhead else 3
    if dtype == "f32r":  # 16KB/partition per tile; cap SBUF use
        rhs_bufs = min(rhs_bufs, 5)

    with tile.TileContext(nc) as tc:
        with (
            tc.tile_pool(name="cpool", bufs=1) as cpool,
            tc.tile_pool(name="gpool", bufs=3) as gpool,
            tc.tile_pool(name="epool", bufs=2) as epool,
            tc.tile_pool(name="persist", bufs=1) as ppool,
            tc.tile_pool(name="tps", bufs=2, space="PSUM") as tpsum,
            tc.tile_pool(name="rhspool", bufs=rhs_bufs) as rhspool,
            tc.tile_pool(name="mpool", bufs=rhs_bufs) as mpool,
            tc.tile_pool(name="outpool", bufs=4) as outpool,
            tc.tile_pool(name="psum", bufs=4, space="PSUM") as psum,
        ):
          for _rep in range(nrep):
            # ---- constants / index preprocessing (on device) ----
            identity = cpool.tile([128, 128], mm_dt)
            make_identity(nc, identity[:])

            hix_sb = cpool.tile([128, NB], I32)
            nc.sync.dma_start(hix_sb[:], hix[:])
            rix_sb = cpool.tile([128, NB], I32)
            nc.sync.dma_start(rix_sb[:], rix[:])

            rf = cpool.tile([128, NB], F32)
            nc.vector.tensor_copy(rf[:], rix_sb[:])
            ge = cpool.tile([128, NB], F32)
            nc.vector.tensor_scalar(
                ge[:], rf[:], float(NUM_REL) - 0.5, None, op0=mybir.AluOpType.is_gt
            )
            # sign s = 1 - 2*[r >= NUM_REL]
            sall = cpool.tile([128, NB], F32)
            nc.vector.tensor_scalar(
                sall[:], ge[:], -2.0, 1.0,
                op0=mybir.AluOpType.mult, op1=mybir.AluOpType.add,
            )
            # r_eff = r - NUM_REL*[r >= NUM_REL]
            ge500 = cpool.tile([128, NB], F32)
            nc.vector.tensor_scalar(
                ge500[:], ge[:], float(NUM_REL), None, op0=mybir.AluOpType.mult
            )
            reff_f = cpool.tile([128, NB], F32)
            nc.vector.tensor_sub(reff_f[:], rf[:], ge500[:])
            reff = cpool.tile([128, NB], I32)
            nc.vector.tensor_copy(reff[:], reff_f[:])

            # ---- gather + elementwise + transpose: build tmpcatT ----
            # tmpcatT[bt][p, c*128 + m] = tmpcat[bt*128 + m, c*128 + p]
            # where tmpcat = [tmp1 | tmp2] over the 1024-wide contraction.
            tmpcatT = [
                ppool.tile([128, KCH * 128], mm_dt, tag=f"tT{b}", name=f"tT{b}")
                for b in range(NB)
            ]

            if not do_prologue:
                scratch = cpool.tile([128, KCH * 128], F32)
                nc.vector.memset(scratch[:], 0.001)
                for b in range(NB):
                    nc.vector.tensor_copy(tmpcatT[b][:], scratch[:])
            for bt in range(NB if do_prologue else 0):
                # one gathered row of ec/rc = [im (512) | re (512)]
                g_e = gpool.tile([128, 2 * DIM], F32, tag="g_e")
                nc.gpsimd.indirect_dma_start(
                    out=g_e[:], out_offset=None, in_=ec[:],
                    in_offset=IndirectOffsetOnAxis(ap=hix_sb[:, bt : bt + 1], axis=0),
                )
                g_r = gpool.tile([128, 2 * DIM], F32, tag="g_r")
                nc.gpsimd.indirect_dma_start(
                    out=g_r[:], out_offset=None, in_=rc[:],
                    in_offset=IndirectOffsetOnAxis(ap=reff[:, bt : bt + 1], axis=0),
                )
                h_im, h_re = g_e[:, :DIM], g_e[:, DIM:]
                r_im, r_re = g_r[:, :DIM], g_r[:, DIM:]

                # r_im' = s * r_im  (per-partition scalar)
                rimp = epool.tile([128, DIM], F32, tag="rimp")
                nc.vector.tensor_scalar(
                    rimp[:], r_im, sall[:, bt : bt + 1], None,
                    op0=mybir.AluOpType.mult,
                )
                tmpc = epool.tile([128, 2 * DIM], mm_dt, tag="tmpc")
                # tmp1 = h_im*r_re + h_re*r_im'
                pa = epool.tile([128, DIM], F32, tag="pa")
                nc.vector.tensor_mul(pa[:], h_im, r_re)
                pb = epool.tile([128, DIM], F32, tag="pb")
                nc.vector.tensor_mul(pb[:], h_re, rimp[:])
                nc.vector.tensor_add(tmpc[:, :DIM], pa[:], pb[:])
                # tmp2 = h_re*r_re - h_im*r_im'
                pc = epool.tile([128, DIM], F32, tag="pc")
                nc.vector.tensor_mul(pc[:], h_re, r_re)
                pd = epool.tile([128, DIM], F32, tag="pd")
                nc.vector.tensor_mul(pd[:], h_im, rimp[:])
                nc.vector.tensor_sub(tmpc[:, DIM:], pc[:], pd[:])

                for c in range(KCH):
                    pt = tpsum.tile([128, 128], mm_dt, tag="pt")
                    nc.tensor.transpose(
                        pt[:], tmpc[:, c * 128 : (c + 1) * 128], identity[:]
                    )
                    nc.vector.tensor_copy(
                        tmpcatT[bt][:, c * 128 : (c + 1) * 128], pt[:]
                    )

            # ---- main GEMM: out[b, e] = tmpcat @ Ecat^T + mask ----
            if not do_rhs_dma:
                rhs_static = rhspool.tile([128, KCH * ET], mm_dt, tag="rhss")
                scratch2 = cpool.tile([128, KCH * ET], F32)
                nc.vector.memset(scratch2[:], 0.001)
                nc.vector.tensor_copy(rhs_static[:], scratch2[:])

            if nhead:
                order = (
                    [(et, bt) for et in range(nhead) for bt in range(NB // 2)]
                    + [(et, bt) for et in range(nhead) for bt in range(NB // 2, NB)]
                    + [(et, bt) for et in range(nhead, N_ET) for bt in range(NB)]
                )
            else:
                order = [(et, bt) for et in range(N_ET) for bt in range(NB)]

            rhs_tiles, mask_tiles = {}, {}
            for et, bt in order:
                e0 = et * ET
                if et not in rhs_tiles:
                    if do_rhs_dma:
                        r_t = rhspool.tile([128, KCH * ET], mm_dt, tag="rhs")
                        nc.sync.dma_start(
                            r_t[:], ecatT[:, et * KCH * ET : (et + 1) * KCH * ET]
                        )
                        rhs_tiles[et] = r_t
                    else:
                        rhs_tiles[et] = rhs_static
                    m_t = mpool.tile([128, ET], F32, tag="mtile")
                    nc.sync.dma_start(m_t[:], maskrep[:, e0 : e0 + ET])
                    mask_tiles[et] = m_t
                rhs = rhs_tiles[et]
                mtile = mask_tiles[et]

                if not do_mm:
                    continue
                ps = psum.tile([128, ET], F32, tag="ps")
                for c in range(KCH):
                    nc.tensor.matmul(
                        ps[:],
                        lhsT=tmpcatT[bt][:, c * 128 : (c + 1) * 128],
                        rhs=rhs[:, c * ET : (c + 1) * ET],
                        start=(c == 0),
                        stop=(c == KCH - 1),
                    )
                if store_all or et == 0:
                    ot = outpool.tile([128, ET], F32, tag="ot")
                    nc.vector.tensor_add(ot[:], ps[:], mtile[:])
                    nc.sync.dma_start(
                        out[bt * 128 : (bt + 1) * 128, e0 : e0 + ET], ot[:]
                    )

    nc.compile()
    return nc


_NC_CACHE = {}


def _get_module(dtype="bf16"):
    if dtype not in _NC_CACHE:
        _NC_CACHE[dtype] = build_module(dtype)
    return _NC_CACHE[dtype]


def make_in_maps(h, r, E_im, E_re, R_im, R_re, mask, dtype="bf16"):
    """Host-side sharding / layout prep."""
    np_mm = mybir.dt.np({"bf16": BF16, "f32r": F32R}[dtype])
    h32 = np.ascontiguousarray(np.asarray(h, dtype=np.int32).reshape(NB, 128).T)
    r32 = np.ascontiguousarray(np.asarray(r, dtype=np.int32).reshape(NB, 128).T)
    E_im = np.asarray(E_im, dtype=np.float32)
    E_re = np.asarray(E_re, dtype=np.float32)
    ec = np.ascontiguousarray(np.concatenate([E_im, E_re], axis=1))
    rc = np.ascontiguousarray(
        np.concatenate(
            [np.asarray(R_im, np.float32)[:NUM_REL], np.asarray(R_re, np.float32)[:NUM_REL]],
            axis=1,
        )
    )
    mask = np.asarray(mask, dtype=np.float32).reshape(1, NUM_ENTITIES)

    in_maps = []
    for k in range(N_CORES):
        sl = slice(k * ESH, (k + 1) * ESH)
        # ecatT[p, ((et*KCH)+c)*ET + j] = Ecat_k[et*ET + j, c*128 + p]
        ecat_k = ec[sl]  # [ESH, 1024] view
        ecatT = np.ascontiguousarray(
            ecat_k.reshape(N_ET, ET, KCH, 128)
            .transpose(3, 0, 2, 1)
            .reshape(128, N_ET * KCH * ET)
            .astype(np_mm)
        )
        in_maps.append(
            {
                "hix": h32,
                "rix": r32,
                "ec": ec,
                "rc": rc,
                "ecatT": ecatT,
                "maskrep": np.ascontiguousarray(
                    np.broadcast_to(mask[:, sl], (128, ESH))
                ),
            }
        )
    return in_maps


def kernel(h, r, E_im, E_re, R_im, R_re, mask):
    nc = _get_module()
    in_maps = make_in_maps(h, r, E_im, E_re, R_im, R_re, mask)
    res = run_bass_kernel_spmd(nc, in_maps, core_ids=list(range(N_CORES)))
    return np.concatenate([res.results[k]["out"] for k in range(N_CORES)], axis=1)
